# revision 8
# baseline (speedup 1.0000x reference)
"""Trainium2 Bass kernel for nn_Attention_43181601194684.

Reference computation:
    h_last  = hidden[0, 1]                          # [B, H]
    proj    = einsum('blh,oh->blo', enc, W) + b     # [B, L, H]
    energies= einsum('bh,blh->bl', h_last, proj)    # [B, L]
    out     = softmax(energies, axis=1)[:, None, :] # [B, 1, L]

Algebraic simplification:
    energies[b, l] = (h_last[b] @ W) . enc[b, l] + (h_last[b] . bias)
The per-batch constant cancels inside the softmax, so the device kernel
computes   e[b, l] = v[b] . enc[b, l]   with v = h_last @ W, followed by a
numerically-stable softmax over l.

v is produced from the tiny [B,H]x[H,H] GEMM on the host and shipped
pre-broadcast ([B_LOC, 128, H], 1 MiB/core) so the device spends zero
critical-path latency on it.  The device is purely the memory-bound part:
stream the 32 MiB/core encoder slice, fused multiply+row-reduce on the DVE,
per-batch softmax (PE cross-partition reductions, ACT exp, DVE scaling).

Sharding: data-parallel over batch. 32 batches / 8 cores = 4 per core.

Layout (every DMA is a fully contiguous DRAM blob):
  - batch 0 is chunked as [2,2,4,8,8,8] l-rows/partition so the first STT
    fires after only 512 KiB; later batches use 2 MiB chunks ([8,8,8,8]).
    Within a chunk of j rows starting at offset `off`: l = off + p*j + k.
  - the [128, 32] per-batch probability tile is stored as-is (contiguous
    16 KiB); the host inverts the per-chunk (p,k) permutation in numpy.
"""

import numpy as np

B, L, H = 32, 4096, 512
N_CORES = 8
B_LOC = B // N_CORES  # 4
P = 128               # SBUF partitions
NCOL = L // P         # 32 energy columns per batch

# per-batch chunk schedules: l-rows per partition for each chunk
SCHED0 = (2, 2, 4, 8, 8, 8)   # batch 0: small head chunks for an early start
SCHED = (8, 8, 8, 8)          # batches 1..3

_PROGRAM = None


def _build_program():
    """Build + compile the single-core Bass/Tile program (SPMD across 8 cores)."""
    from contextlib import ExitStack

    import concourse.bacc as bacc
    import concourse.mybir as mybir
    import concourse.tile as tile
    from concourse.masks import make_identity

    fp32 = mybir.dt.float32
    Alu = mybir.AluOpType
    Act = mybir.ActivationFunctionType

    nc = bacc.Bacc("TRN2", target_bir_lowering=False, debug=False,
                   num_devices=N_CORES)

    enc = nc.dram_tensor("enc", [B_LOC, L, H], fp32, kind="ExternalInput")
    vr = nc.dram_tensor("vr", [B_LOC, P, H], fp32, kind="ExternalInput")
    probs = nc.dram_tensor("probs", [B_LOC, P, NCOL], fp32,
                           kind="ExternalOutput")

    # one rearranged view per chunk-row-count; chunk g of the k=j view
    # covers l in [g*128*j, (g+1)*128*j) with l = off + p*j + k
    enc_r = {
        j: enc.rearrange("b (g p k) h -> b g p k h", p=P, k=j)
        for j in (2, 4, 8)
    }

    with tile.TileContext(nc) as tc, ExitStack() as ctx:
        consts = ctx.enter_context(tc.tile_pool(name="consts", bufs=1))
        wpool = ctx.enter_context(tc.tile_pool(name="wpool", bufs=1))
        et8 = ctx.enter_context(tc.tile_pool(name="et8", bufs=8))
        et4 = ctx.enter_context(tc.tile_pool(name="et4", bufs=1))
        et2 = ctx.enter_context(tc.tile_pool(name="et2", bufs=2))
        scratch = ctx.enter_context(tc.tile_pool(name="scratch", bufs=2))
        epers = ctx.enter_context(tc.tile_pool(name="epers", bufs=1))
        small = ctx.enter_context(tc.tile_pool(name="small", bufs=2))
        psum = ctx.enter_context(tc.tile_pool(name="psum", bufs=2, space="PSUM"))
        etp = {2: et2, 4: et4, 8: et8}

        ndma = [0]

        def ring():
            ndma[0] += 1
            return nc.scalar if ndma[0] % 2 else nc.sync

        # priority block: v[0] + the two 512 KiB head chunks of batch 0 land
        # before anything else so the first STT fires as early as possible
        v_sb = {}
        head = {}
        with tc.high_priority():
            v_sb[0] = wpool.tile([P, H], fp32, tag="v0", name="v0")
            nc.sync.dma_start(v_sb[0][:], vr[0])
            for g in range(2):
                t = et2.tile([P, 2, H], fp32, tag="et2", name=f"hd{g}")
                nc.scalar.dma_start(t[:], enc_r[2][0, g])
                head[g] = t
            identity = consts.tile([P, P], fp32, tag="identity")
            make_identity(nc, identity)
            ones_row = consts.tile([1, P], fp32, tag="ones_row")  # bcast lhsT
            nc.vector.memset(ones_row[:], 1.0)
            # all-ones [128,128]: partition-sum WITH broadcast in one matmul
            ones_sq = consts.tile([P, P], fp32, tag="ones_sq")
            nc.vector.memset(ones_sq[:], 1.0)
            for bi in range(1, B_LOC):
                v_sb[bi] = wpool.tile([P, H], fp32, tag=f"v{bi}",
                                      name=f"v{bi}")
                nc.scalar.dma_start(v_sb[bi][:], vr[bi])

        # ---- main stream: energies via fused multiply+row-reduce on DVE ----
        for bi in range(B_LOC):
            sched = SCHED0 if bi == 0 else SCHED
            e_sb = epers.tile([P, NCOL], fp32, tag=f"e{bi}")
            m = 0
            off_rows = 0
            for cix, j in enumerate(sched):
                g = off_rows // j          # group index in the k=j view
                if bi == 0 and cix < 2:
                    et = head[cix]
                else:
                    et = etp[j].tile([P, j, H], fp32, tag=f"et{j}",
                                     name=f"et_{bi}_{cix}")
                    ring().dma_start(et[:], enc_r[j][bi, g])
                for k in range(j):
                    sc = scratch.tile([P, H], fp32, tag="ttr")
                    # fused (enc * v) + row-sum in one native DVE op:
                    # out = (in0 * 1.0) * in1 ; accum_out = row_sum(out)
                    nc.vector.scalar_tensor_tensor(
                        out=sc[:], in0=et[:, k, :], scalar=1.0,
                        in1=v_sb[bi][:],
                        op0=Alu.mult, op1=Alu.mult,
                        accum_out=e_sb[:, m:m + 1],
                    )
                    m += 1
                off_rows += j

            # ---- softmax over the 4096 energies of batch bi ----
            # ACT runs ONLY Exp (avoids 1.3us activation-table reloads);
            # copies/scales go to DVE, cross-partition reductions to PE.
            mx = small.tile([P, 1], fp32, tag="mx")
            nc.vector.tensor_reduce(mx[:], e_sb[:], axis=mybir.AxisListType.X,
                                    op=Alu.max)
            mxT_ps = psum.tile([1, P], fp32, tag="red_ps")
            nc.tensor.transpose(mxT_ps[:], mx[:], identity[:])
            ngmax = small.tile([1, 1], fp32, tag="ngmax")
            nc.vector.tensor_reduce(ngmax[:], mxT_ps[:],
                                    axis=mybir.AxisListType.X, op=Alu.max,
                                    negate=True)
            nb_ps = psum.tile([P, 1], fp32, tag="bc_ps")
            nc.tensor.matmul(nb_ps[:], ones_row[:], ngmax[:],
                             start=True, stop=True)
            nbias = small.tile([P, 1], fp32, tag="nbias")
            nc.vector.tensor_copy(nbias[:], nb_ps[:])

            p_sb = epers.tile([P, NCOL], fp32, tag=f"p{bi}")
            ssum = small.tile([P, 1], fp32, tag="ssum")
            nc.scalar.activation(p_sb[:], e_sb[:], Act.Exp,
                                 bias=nbias[:], scale=1.0, accum_out=ssum[:])

            # partition-sum AND broadcast in one matmul: out[m,0] = sum_p ssum
            tot_ps = psum.tile([P, 1], fp32, tag="red_ps")
            nc.tensor.matmul(tot_ps[:], ones_sq[:], ssum[:],
                             start=True, stop=True)
            rbc = small.tile([P, 1], fp32, tag="rbc")
            nc.vector.reciprocal(rbc[:], tot_ps[:])

            o_sb = epers.tile([P, NCOL], fp32, tag=f"o{bi}")
            nc.vector.tensor_scalar(out=o_sb[:], in0=p_sb[:],
                                    scalar1=rbc[:], scalar2=None,
                                    op0=Alu.mult)

            # contiguous 16 KiB store; host inverts the (p,k) permutation
            nc.scalar.dma_start(probs[bi], o_sb[:])

    nc.compile()
    return nc


def _get_program():
    global _PROGRAM
    if _PROGRAM is None:
        _PROGRAM = _build_program()
    return _PROGRAM


def _core_inputs(enc, v):
    """Per-core input dicts: enc batch-slice + pre-broadcast v tile."""
    in_maps = []
    for core in range(N_CORES):
        b0 = core * B_LOC
        v_rep = np.ascontiguousarray(
            np.broadcast_to(v[b0:b0 + B_LOC][:, None, :], (B_LOC, P, H)),
            dtype=np.float32)
        in_maps.append({
            "enc": np.ascontiguousarray(enc[b0:b0 + B_LOC]),
            "vr": v_rep,
        })
    return in_maps


def _assemble(probs_list):
    """[B_LOC, P, NCOL] per core -> full [B, 1, L].

    Column block [mc, mc+j) of a batch holds chunk (off, j) with
    l = off + p*j + k; flattening [:, mc:mc+j] in C-order (p major,
    k minor) is exactly l-order for that chunk.
    """
    out = np.empty((B, L), dtype=np.float32)
    for core, pr in enumerate(probs_list):
        for bl in range(B_LOC):
            b = core * B_LOC + bl
            sched = SCHED0 if bl == 0 else SCHED
            mc = 0
            off = 0
            for j in sched:
                n = P * j
                out[b, off:off + n] = np.asarray(
                    pr[bl][:, mc:mc + j]).reshape(n)
                mc += j
                off += n
    return out[:, None, :]


def kernel(hidden, encoder_outputs, W, b):
    """Full-input entry point: shards across 8 NeuronCores, returns [B,1,L]."""
    from concourse.bass_utils import run_bass_kernel_spmd

    hidden = np.asarray(hidden, dtype=np.float32)
    enc = np.asarray(encoder_outputs, dtype=np.float32)
    W = np.asarray(W, dtype=np.float32)

    h_last = hidden[0, 1]          # == hidden[0].transpose(1,0,2)[:, -1, :]
    v = (h_last @ W).astype(np.float32)  # [B, H]; bias cancels in softmax

    nc = _get_program()
    in_maps = _core_inputs(enc, v)
    res = run_bass_kernel_spmd(nc, in_maps, list(range(N_CORES)))
    return _assemble([res.results[i]["probs"] for i in range(N_CORES)])


# revision 12
# speedup vs baseline: 1.0624x; 1.0624x over previous
"""Trainium2 Bass kernel for nn_Attention_43181601194684.

Reference computation:
    h_last  = hidden[0, 1]                          # [B, H]
    proj    = einsum('blh,oh->blo', enc, W) + b     # [B, L, H]
    energies= einsum('bh,blh->bl', h_last, proj)    # [B, L]
    out     = softmax(energies, axis=1)[:, None, :] # [B, 1, L]

Algebraic simplification:
    energies[b, l] = (h_last[b] @ W) . enc[b, l] + (h_last[b] . bias)
The per-batch constant cancels inside the softmax, so the device kernel
computes   e[b, l] = v[b] . enc[b, l]   with v = h_last @ W, followed by a
numerically-stable softmax over l.

v = h_last @ W is the tiny [B,H]x[H,H] GEMM, done on the host; the device
receives the 8 KiB v rows and broadcasts each to 128 partitions with a
one-hot PE matmul (kept in PSUM; the DVE reads it from there directly).

The wall time is  queue_start + total_bytes/DMA_BW + tail,  so the design
minimizes bytes (no replicated tensors), keeps every DMA a fully contiguous
DRAM blob with 16 KiB/partition descriptors, limits in-flight chunks so the
descriptor round-robin doesn't starve the serial consumer, and puts the
SMALL chunks at the end of the stream so the post-last-byte tail is short.

Sharding: data-parallel over batch. 32 batches / 8 cores = 4 per core.
Chunk schedule (l-rows per partition): batch 0 = [2,2,4,8,8,8] (early
start), batch 3 = [8,8,8,4,2,2] (short tail), middle batches = [8,8,8,8].
Within a chunk of j rows at row-offset off: l = off + p*j + k.
Output: [128, 32] per-batch tile stored contiguously; host un-permutes.
"""

import numpy as np

B, L, H = 32, 4096, 512
N_CORES = 8
B_LOC = B // N_CORES  # 4
P = 128               # SBUF partitions
NCOL = L // P         # 32 energy columns per batch

SCHEDS = {
    0: (2, 2, 4, 8, 8, 8),
    1: (8, 8, 8, 8),
    2: (8, 8, 8, 8),
    3: (8, 8, 8, 4, 2, 2),
}

_PROGRAM = None


def _build_program():
    """Build + compile the single-core Bass/Tile program (SPMD across 8 cores)."""
    from contextlib import ExitStack

    import concourse.bacc as bacc
    import concourse.mybir as mybir
    import concourse.tile as tile
    from concourse.masks import make_identity

    fp32 = mybir.dt.float32
    Alu = mybir.AluOpType
    Act = mybir.ActivationFunctionType

    nc = bacc.Bacc("TRN2", target_bir_lowering=False, debug=False,
                   num_devices=N_CORES)

    enc = nc.dram_tensor("enc", [B_LOC, L, H], fp32, kind="ExternalInput")
    v4d = nc.dram_tensor("v4", [B_LOC, H], fp32, kind="ExternalInput")
    probs = nc.dram_tensor("probs", [B_LOC, P, NCOL], fp32,
                           kind="ExternalOutput")

    # one rearranged view per chunk-row-count; chunk g of the k=j view
    # covers l in [g*128*j, (g+1)*128*j) with l = g*128*j + p*j + k
    enc_r = {
        j: enc.rearrange("b (g p k) h -> b g p k h", p=P, k=j)
        for j in (2, 4, 8)
    }

    with tile.TileContext(nc) as tc, ExitStack() as ctx:
        consts = ctx.enter_context(tc.tile_pool(name="consts", bufs=1))
        wpool = ctx.enter_context(tc.tile_pool(name="wpool", bufs=1))
        et8 = ctx.enter_context(tc.tile_pool(name="et8", bufs=4))
        et4 = ctx.enter_context(tc.tile_pool(name="et4", bufs=2))
        et2 = ctx.enter_context(tc.tile_pool(name="et2", bufs=4))
        scratch = ctx.enter_context(tc.tile_pool(name="scratch", bufs=2,
                                                 space="PSUM"))
        vbp = ctx.enter_context(tc.tile_pool(name="vbp", bufs=1, space="PSUM"))
        epers = ctx.enter_context(tc.tile_pool(name="epers", bufs=1))
        small = ctx.enter_context(tc.tile_pool(name="small", bufs=2))
        psum = ctx.enter_context(tc.tile_pool(name="psum", bufs=1, space="PSUM"))
        etp = {2: et2, 4: et4, 8: et8}

        ndma = [0]

        def ring():
            ndma[0] += 1
            return nc.scalar if ndma[0] % 2 else nc.sync

        # priority block: the tiny v tensor plus batch 0's head chunks land
        # first; v is broadcast to all partitions via one-hot PE matmuls and
        # stays resident in PSUM (STT reads in1 from PSUM directly).
        head = {}
        v_bc = {}
        with tc.high_priority():
            v_sb4 = wpool.tile([B_LOC, H], fp32, tag="v4")
            nc.sync.dma_start(v_sb4[:], v4d[:])
            for g in range(2):
                t = et2.tile([P, 2, H], fp32, tag="et2", name=f"hd{g}")
                nc.scalar.dma_start(t[:], enc_r[2][0, g])
                head[g] = t
            identity = consts.tile([P, P], fp32, tag="identity")
            make_identity(nc, identity)
            ones_row = consts.tile([1, P], fp32, tag="ones_row")  # bcast lhsT
            nc.vector.memset(ones_row[:], 1.0)
            # all-ones [128,128]: partition-sum WITH broadcast in one matmul
            ones_sq = consts.tile([P, P], fp32, tag="ones_sq")
            nc.vector.memset(ones_sq[:], 1.0)

            def bcast_v(bi):
                # sel[k, m] = (k == bi)  =>  out[m, :] = v4[bi, :] for all m
                sel = consts.tile([B_LOC, P], fp32, tag=f"sel{bi}",
                                  name=f"sel{bi}")
                nc.gpsimd.memset(sel[:], 0.0)
                nc.gpsimd.affine_select(
                    out=sel[:], in_=sel[:], compare_op=Alu.not_equal,
                    fill=1.0, base=-bi, pattern=[[0, P]],
                    channel_multiplier=1,
                )
                vb = vbp.tile([P, H], fp32, tag=f"vb{bi}", name=f"vb{bi}")
                nc.tensor.matmul(vb[:], sel[:], v_sb4[:],
                                 start=True, stop=True)
                return vb

            v_bc[0] = bcast_v(0)

        # ---- main stream: energies via fused multiply+row-reduce on DVE ----
        for bi in range(B_LOC):
            if bi not in v_bc:
                v_bc[bi] = bcast_v(bi)
            sched = SCHEDS[bi]
            e_sb = epers.tile([P, NCOL], fp32, tag=f"e{bi}")
            m = 0
            off_rows = 0
            for cix, j in enumerate(sched):
                g = off_rows // j          # group index in the k=j view
                if bi == 0 and cix < 2:
                    et = head[cix]
                else:
                    et = etp[j].tile([P, j, H], fp32, tag=f"et{j}",
                                     name=f"et_{bi}_{cix}")
                    ring().dma_start(et[:], enc_r[j][bi, g])
                for k in range(j):
                    sc = scratch.tile([P, H], fp32, tag="ttr")
                    # fused (enc * v) + row-sum in one native DVE op:
                    # out = (in0 * 1.0) * in1 ; accum_out = row_sum(out)
                    nc.vector.scalar_tensor_tensor(
                        out=sc[:], in0=et[:, k, :], scalar=1.0,
                        in1=v_bc[bi][:],
                        op0=Alu.mult, op1=Alu.mult,
                        accum_out=e_sb[:, m:m + 1],
                    )
                    m += 1
                off_rows += j

            # ---- softmax over the 4096 energies of batch bi ----
            # ACT runs ONLY Exp (an activation-table reload costs 1.3us);
            # tiny copies/scales go to gpsimd, cross-partition work to PE.
            mx = small.tile([P, 1], fp32, tag="mx")
            nc.vector.tensor_reduce(mx[:], e_sb[:], axis=mybir.AxisListType.X,
                                    op=Alu.max)
            mxT_ps = psum.tile([1, P], fp32, tag="red_ps")
            nc.tensor.transpose(mxT_ps[:], mx[:], identity[:])
            ngmax = small.tile([1, 1], fp32, tag="ngmax")
            nc.vector.tensor_reduce(ngmax[:], mxT_ps[:],
                                    axis=mybir.AxisListType.X, op=Alu.max,
                                    negate=True)
            nb_ps = psum.tile([P, 1], fp32, tag="bc_ps")
            nc.tensor.matmul(nb_ps[:], ones_row[:], ngmax[:],
                             start=True, stop=True)
            nbias = small.tile([P, 1], fp32, tag="nbias")
            nc.vector.tensor_copy(nbias[:], nb_ps[:])

            p_sb = epers.tile([P, NCOL], fp32, tag=f"p{bi}")
            ssum = small.tile([P, 1], fp32, tag="ssum")
            nc.scalar.activation(p_sb[:], e_sb[:], Act.Exp,
                                 bias=nbias[:], scale=1.0, accum_out=ssum[:])

            # partition-sum AND broadcast in one matmul: out[m,0] = sum_p ssum
            tot_ps = psum.tile([P, 1], fp32, tag="red_ps")
            nc.tensor.matmul(tot_ps[:], ones_sq[:], ssum[:],
                             start=True, stop=True)
            rbc = small.tile([P, 1], fp32, tag="rbc")
            nc.vector.reciprocal(rbc[:], tot_ps[:])

            o_sb = epers.tile([P, NCOL], fp32, tag=f"o{bi}")
            nc.vector.tensor_scalar(out=o_sb[:], in0=p_sb[:],
                                    scalar1=rbc[:], scalar2=None,
                                    op0=Alu.mult)

            # contiguous 16 KiB store; host inverts the (p,k) permutation
            nc.scalar.dma_start(probs[bi], o_sb[:])

    nc.compile()
    return nc


def _get_program():
    global _PROGRAM
    if _PROGRAM is None:
        _PROGRAM = _build_program()
    return _PROGRAM


def _core_inputs(enc, v):
    """Per-core input dicts: enc batch-slice + that core's v rows."""
    in_maps = []
    for core in range(N_CORES):
        b0 = core * B_LOC
        in_maps.append({
            "enc": np.ascontiguousarray(enc[b0:b0 + B_LOC]),
            "v4": np.ascontiguousarray(v[b0:b0 + B_LOC]),
        })
    return in_maps


def _assemble(probs_list):
    """[B_LOC, P, NCOL] per core -> full [B, 1, L].

    Column block [mc, mc+j) of a batch holds chunk (off, j) with
    l = off + p*j + k; flattening [:, mc:mc+j] in C-order (p major,
    k minor) is exactly l-order for that chunk.
    """
    out = np.empty((B, L), dtype=np.float32)
    for core, pr in enumerate(probs_list):
        for bl in range(B_LOC):
            b = core * B_LOC + bl
            mc = 0
            off = 0
            for j in SCHEDS[bl]:
                n = P * j
                out[b, off:off + n] = np.asarray(
                    pr[bl][:, mc:mc + j]).reshape(n)
                mc += j
                off += n
    return out[:, None, :]


def kernel(hidden, encoder_outputs, W, b):
    """Full-input entry point: shards across 8 NeuronCores, returns [B,1,L]."""
    from concourse.bass_utils import run_bass_kernel_spmd

    hidden = np.asarray(hidden, dtype=np.float32)
    enc = np.asarray(encoder_outputs, dtype=np.float32)
    W = np.asarray(W, dtype=np.float32)

    h_last = hidden[0, 1]          # == hidden[0].transpose(1,0,2)[:, -1, :]
    v = (h_last @ W).astype(np.float32)  # [B, H]; bias cancels in softmax

    nc = _get_program()
    in_maps = _core_inputs(enc, v)
    res = run_bass_kernel_spmd(nc, in_maps, list(range(N_CORES)))
    return _assemble([res.results[i]["probs"] for i in range(N_CORES)])


# revision 13
# speedup vs baseline: 1.3317x; 1.2536x over previous
"""Trainium2 Bass kernel for nn_Attention_43181601194684.

Reference computation:
    h_last  = hidden[0, 1]                          # [B, H]
    proj    = einsum('blh,oh->blo', enc, W) + b     # [B, L, H]
    energies= einsum('bh,blh->bl', h_last, proj)    # [B, L]
    out     = softmax(energies, axis=1)[:, None, :] # [B, 1, L]

Algebraic simplification:
    energies[b, l] = (h_last[b] @ W) . enc[b, l] + (h_last[b] . bias)
The per-batch constant cancels inside the softmax, so the device kernel
computes   e[b, l] = v[b] . enc[b, l]   with v = h_last @ W, followed by a
numerically-stable softmax over l.  v = h_last @ W (the tiny [B,H]x[H,H]
GEMM) is done on the host.

Precision: enc and v are streamed in FP16 (host-converted); the products
accumulate into FP32 energies and the softmax runs in FP32.  Measured
output rel-err vs the fp32 reference is ~5e-3 (gate is 2e-2).  FP16 halves
the HBM traffic (16.8 MiB/core) AND doubles DVE throughput (2x_1p mode).

The wall time is  queue_start + total_bytes/DMA_BW + tail,  so the design
minimizes bytes, keeps every DMA a fully contiguous DRAM blob with fat
per-partition descriptor runs, limits in-flight chunks so the descriptor
round-robin doesn't starve the serial consumer, and puts SMALL chunks at
the two ends of the stream (early first STT, short post-last-byte tail).

Sharding: data-parallel over batch. 32 batches / 8 cores = 4 per core.
Chunk schedule (l-rows per partition; a 16-row fp16 chunk is 2 MiB):
batch 0 = [2,2,4,8,16], batches 1,2 = [16,16], batch 3 = [16,8,4,2,2].
Within a chunk of j rows at row-offset off: l = off + p*j + k.
Output: [128, 32] fp32 per-batch tile stored contiguously; host un-permutes.
"""

import numpy as np

B, L, H = 32, 4096, 512
N_CORES = 8
B_LOC = B // N_CORES  # 4
P = 128               # SBUF partitions
NCOL = L // P         # 32 energy columns per batch

SCHEDS = {
    0: (2, 2, 4, 8, 16),
    1: (16, 16),
    2: (16, 16),
    3: (16, 8, 4, 2, 2),
}

_PROGRAM = None


def _build_program():
    """Build + compile the single-core Bass/Tile program (SPMD across 8 cores)."""
    from contextlib import ExitStack

    import concourse.bacc as bacc
    import concourse.mybir as mybir
    import concourse.tile as tile
    from concourse.masks import make_identity

    fp32 = mybir.dt.float32
    fp16 = mybir.dt.float16
    Alu = mybir.AluOpType
    Act = mybir.ActivationFunctionType

    nc = bacc.Bacc("TRN2", target_bir_lowering=False, debug=False,
                   num_devices=N_CORES)

    enc = nc.dram_tensor("enc", [B_LOC, L, H], fp16, kind="ExternalInput")
    vr = nc.dram_tensor("vr", [B_LOC, P, H], fp16, kind="ExternalInput")
    probs = nc.dram_tensor("probs", [B_LOC, P, NCOL], fp32,
                           kind="ExternalOutput")

    # one rearranged view per chunk-row-count; chunk g of the k=j view
    # covers l in [g*128*j, (g+1)*128*j) with l = g*128*j + p*j + k
    enc_r = {
        j: enc.rearrange("b (g p k) h -> b g p k h", p=P, k=j)
        for j in (2, 4, 8, 16)
    }

    with tile.TileContext(nc) as tc, ExitStack() as ctx:
        consts = ctx.enter_context(tc.tile_pool(name="consts", bufs=1))
        wpool = ctx.enter_context(tc.tile_pool(name="wpool", bufs=1))
        et16 = ctx.enter_context(tc.tile_pool(name="et16", bufs=3))
        et8 = ctx.enter_context(tc.tile_pool(name="et8", bufs=2))
        et4 = ctx.enter_context(tc.tile_pool(name="et4", bufs=2))
        et2 = ctx.enter_context(tc.tile_pool(name="et2", bufs=4))
        scratch = ctx.enter_context(tc.tile_pool(name="scratch", bufs=2))
        epers = ctx.enter_context(tc.tile_pool(name="epers", bufs=1))
        small = ctx.enter_context(tc.tile_pool(name="small", bufs=2))
        psum = ctx.enter_context(tc.tile_pool(name="psum", bufs=2, space="PSUM"))
        etp = {2: et2, 4: et4, 8: et8, 16: et16}

        ndma = [0]

        def ring():
            ndma[0] += 1
            return nc.scalar if ndma[0] % 2 else nc.sync

        # priority block: v (replicated fp16, 512 KiB) plus batch 0's head
        # chunks land first so the first STT fires as early as possible
        head = {}
        v_sb = {}
        with tc.high_priority():
            v_sb[0] = wpool.tile([P, H], fp16, tag="v0", name="v0")
            nc.sync.dma_start(v_sb[0][:], vr[0])
            for g in range(2):
                t = et2.tile([P, 2, H], fp16, tag="et2", name=f"hd{g}")
                nc.scalar.dma_start(t[:], enc_r[2][0, g])
                head[g] = t
            identity = consts.tile([P, P], fp32, tag="identity")
            make_identity(nc, identity)
            ones_row = consts.tile([1, P], fp32, tag="ones_row")  # bcast lhsT
            nc.vector.memset(ones_row[:], 1.0)
            # all-ones [128,128]: partition-sum WITH broadcast in one matmul
            ones_sq = consts.tile([P, P], fp32, tag="ones_sq")
            nc.vector.memset(ones_sq[:], 1.0)
            for bi in range(1, B_LOC):
                v_sb[bi] = wpool.tile([P, H], fp16, tag=f"v{bi}",
                                      name=f"v{bi}")
                nc.scalar.dma_start(v_sb[bi][:], vr[bi])

        # ---- main stream: energies via fused multiply+row-reduce on DVE ----
        for bi in range(B_LOC):
            sched = SCHEDS[bi]
            e_sb = epers.tile([P, NCOL], fp32, tag=f"e{bi}")
            m = 0
            off_rows = 0
            for cix, j in enumerate(sched):
                g = off_rows // j          # group index in the k=j view
                if bi == 0 and cix < 2:
                    et = head[cix]
                else:
                    et = etp[j].tile([P, j, H], fp16, tag=f"et{j}",
                                     name=f"et_{bi}_{cix}")
                    ring().dma_start(et[:], enc_r[j][bi, g])
                for k in range(j):
                    sc = scratch.tile([P, H], fp16, tag="ttr")
                    # fused (enc * v) + row-sum in one native DVE op, fp16
                    # operands -> 2x_1p mode; accum_out stays fp32
                    nc.vector.scalar_tensor_tensor(
                        out=sc[:], in0=et[:, k, :], scalar=1.0,
                        in1=v_sb[bi][:],
                        op0=Alu.mult, op1=Alu.mult,
                        accum_out=e_sb[:, m:m + 1],
                    )
                    m += 1
                off_rows += j

            # ---- softmax over the 4096 energies of batch bi (fp32) ----
            # ACT runs ONLY Exp (an activation-table reload costs 1.3us);
            # tiny copies/scales go to DVE, cross-partition work to PE.
            mx = small.tile([P, 1], fp32, tag="mx")
            nc.vector.tensor_reduce(mx[:], e_sb[:], axis=mybir.AxisListType.X,
                                    op=Alu.max)
            mxT_ps = psum.tile([1, P], fp32, tag="red_ps")
            nc.tensor.transpose(mxT_ps[:], mx[:], identity[:])
            ngmax = small.tile([1, 1], fp32, tag="ngmax")
            nc.vector.tensor_reduce(ngmax[:], mxT_ps[:],
                                    axis=mybir.AxisListType.X, op=Alu.max,
                                    negate=True)
            nb_ps = psum.tile([P, 1], fp32, tag="bc_ps")
            nc.tensor.matmul(nb_ps[:], ones_row[:], ngmax[:],
                             start=True, stop=True)
            nbias = small.tile([P, 1], fp32, tag="nbias")
            nc.vector.tensor_copy(nbias[:], nb_ps[:])

            p_sb = epers.tile([P, NCOL], fp32, tag=f"p{bi}")
            ssum = small.tile([P, 1], fp32, tag="ssum")
            nc.scalar.activation(p_sb[:], e_sb[:], Act.Exp,
                                 bias=nbias[:], scale=1.0, accum_out=ssum[:])

            # partition-sum AND broadcast in one matmul: out[m,0] = sum_p ssum
            tot_ps = psum.tile([P, 1], fp32, tag="red_ps")
            nc.tensor.matmul(tot_ps[:], ones_sq[:], ssum[:],
                             start=True, stop=True)
            rbc = small.tile([P, 1], fp32, tag="rbc")
            nc.vector.reciprocal(rbc[:], tot_ps[:])

            o_sb = epers.tile([P, NCOL], fp32, tag=f"o{bi}")
            nc.vector.tensor_scalar(out=o_sb[:], in0=p_sb[:],
                                    scalar1=rbc[:], scalar2=None,
                                    op0=Alu.mult)

            # contiguous 16 KiB store; host inverts the (p,k) permutation
            nc.scalar.dma_start(probs[bi], o_sb[:])

    nc.compile()
    return nc


def _get_program():
    global _PROGRAM
    if _PROGRAM is None:
        _PROGRAM = _build_program()
    return _PROGRAM


def _core_inputs(enc, v):
    """Per-core input dicts: fp16 enc batch-slice + replicated fp16 v."""
    enc16 = enc.astype(np.float16)
    v16 = v.astype(np.float16)
    in_maps = []
    for core in range(N_CORES):
        b0 = core * B_LOC
        v_rep = np.ascontiguousarray(
            np.broadcast_to(v16[b0:b0 + B_LOC][:, None, :], (B_LOC, P, H)))
        in_maps.append({
            "enc": np.ascontiguousarray(enc16[b0:b0 + B_LOC]),
            "vr": v_rep,
        })
    return in_maps


def _assemble(probs_list):
    """[B_LOC, P, NCOL] per core -> full [B, 1, L].

    Column block [mc, mc+j) of a batch holds chunk (off, j) with
    l = off + p*j + k; flattening [:, mc:mc+j] in C-order (p major,
    k minor) is exactly l-order for that chunk.
    """
    out = np.empty((B, L), dtype=np.float32)
    for core, pr in enumerate(probs_list):
        for bl in range(B_LOC):
            b = core * B_LOC + bl
            mc = 0
            off = 0
            for j in SCHEDS[bl]:
                n = P * j
                out[b, off:off + n] = np.asarray(
                    pr[bl][:, mc:mc + j]).reshape(n)
                mc += j
                off += n
    return out[:, None, :]


def kernel(hidden, encoder_outputs, W, b):
    """Full-input entry point: shards across 8 NeuronCores, returns [B,1,L]."""
    from concourse.bass_utils import run_bass_kernel_spmd

    hidden = np.asarray(hidden, dtype=np.float32)
    enc = np.asarray(encoder_outputs, dtype=np.float32)
    W = np.asarray(W, dtype=np.float32)

    h_last = hidden[0, 1]          # == hidden[0].transpose(1,0,2)[:, -1, :]
    v = (h_last @ W).astype(np.float32)  # [B, H]; bias cancels in softmax

    nc = _get_program()
    in_maps = _core_inputs(enc, v)
    res = run_bass_kernel_spmd(nc, in_maps, list(range(N_CORES)))
    return _assemble([res.results[i]["probs"] for i in range(N_CORES)])
